# revision 9
# baseline (speedup 1.0000x reference)
"""Trainium2 Bass kernel: batched 64-digit base-10 addition (nn_Adder).

The reference RNN scan is carry-propagating decimal addition. The DVE
scan instruction is the only engine that can walk the carry recurrence,
and it runs at a fixed ~2.15 ns/element regardless of dtype — so the
kernel works in radix-10^4: each group of FOUR digits is one limb,
quartering the scan length per row (16 limbs instead of 64 digits).

Per core (pure data parallel across 8 cores, batch 524288 -> 65536 rows):

  * Inputs are uploaded as fp8e4 (digits 0-9 exact in e4m3): 4x less
    input HBM traffic than f32. Output leaves as one uint16 per FOUR
    digits (the raw scan state v = limb + 10^4*carry <= 19999, exact in
    u16): 8x less output traffic. The host decodes v % 10^4 into digit
    columns with numpy divmods.
  * Stage 1 (PE, fp8 DoubleRow perf mode): one DoubleRow matmul per
    source computes 10*d_even + d_odd for every digit pair — the
    weights [10I | I] pair with strided views of the even/odd digit
    positions (k-tile dim), and the MSB->LSB reversal is folded into
    the same access pattern. PSUM gets base-100 limbs M <= 198,
    LSB-first.
  * ACT drains M to SBUF as bf16 (integers <= 256 exact in bf16).
  * Stage 2 (PE, bf16): two accumulating matmuls with weights 100I / I
    over the odd/even base-100 limbs -> PSUM radix-10^4 limbs <= 19998.
  * DVE runs the whole carry chain in ONE scan per tile, reading PSUM:
    v_t = [10^4 <= v_{t-1}] + s_t. Row boundaries are killed by the
    data0 pattern operand (3e9 at each row's first limb). The scan
    writes the u16 OUTPUT tile directly - zero post-processing.
  * A burst of dummy matmuls right after the weight DMAs ramps the PE
    pstate (full clock needs ~3us of continuous execution) before the
    first data tile lands.
  * Small first/last tiles (G_LIST) shorten pipeline fill and drain.
  * GpSimd untouched (it would steal the DVE SBUF ports).

All intermediate values are small integers, exact in fp8/bf16/f32/u16 ->
bit-exact output after the host-side decode.
"""

import sys

sys.path.insert(0, "/opt/trn_rl_repo")

import numpy as np

BATCH = 524288
SEQ = 64
N_CORES = 8
B_LOC = BATCH // N_CORES

P = 128
LIMBS2 = SEQ // 2   # 32 base-100 limbs per row (stage-1)
LIMBS4 = SEQ // 4   # 16 base-10^4 limbs per row (stage-2 / scan / output)
G_LIST = [8, 8, 16] + [32] * 14 + [16, 8, 8]
G_MAX = max(G_LIST)
T = len(G_LIST)
FDM = G_MAX * SEQ       # padded digit cols (a/b tiles)
F2M = G_MAX * LIMBS2    # padded base-100 limb cols
F4M = G_MAX * LIMBS4    # padded base-10^4 limb cols
MW = 512                # matmul window = one PSUM bank of f32
WARMUP_MM = 24

IO_BUFS = T             # all input DMAs queued up front
WK_BUFS = 4

_nc_cache = {}


def _build_adder():
    from contextlib import ExitStack

    import concourse.bacc as bacc
    import concourse.mybir as mybir
    import concourse.tile as tile

    F32 = mybir.dt.float32
    BF16 = mybir.dt.bfloat16
    FP8 = mybir.dt.float8e4
    U16 = mybir.dt.uint16
    ALU = mybir.AluOpType
    DR = mybir.MatmulPerfMode.DoubleRow

    assert P * sum(G_LIST) == B_LOC

    nc = bacc.Bacc("TRN2", target_bir_lowering=False, debug=False)
    a_ext = nc.declare_dram_parameter("a", [B_LOC, SEQ], FP8, isOutput=False)
    b_ext = nc.declare_dram_parameter("b", [B_LOC, SEQ], FP8, isOutput=False)
    # [10I | I] fp8 pair-weights for DoubleRow stage 1
    edr_ext = nc.declare_dram_parameter("eyedr", [P, 2 * P], FP8,
                                        isOutput=False)
    # 100I and I in bf16 for stage 2
    e100_ext = nc.declare_dram_parameter("eye100", [P, P], BF16,
                                         isOutput=False)
    e1_ext = nc.declare_dram_parameter("eye1", [P, P], BF16, isOutput=False)
    o_ext = nc.declare_dram_parameter("out", [B_LOC, LIMBS4], U16,
                                      isOutput=True)

    with tile.TileContext(nc) as tc, ExitStack() as ctx:
        cpool = ctx.enter_context(tc.tile_pool(name="const", bufs=1))
        # scan data0: 10^4 everywhere, "+inf" at each row's first limb so
        # the carry chain resets at row boundaries within a partition
        pat = cpool.tile([P, F4M], F32)
        nc.vector.memset(pat[:], 10000.0)
        nc.vector.memset(pat[:, 0:F4M:LIMBS4], 3.0e9)
        edr = cpool.tile([P, 2 * P], FP8)
        e100 = cpool.tile([P, P], BF16)
        e1 = cpool.tile([P, P], BF16)
        nc.sync.dma_start(out=edr[:], in_=edr_ext[:])
        nc.sync.dma_start(out=e100[:], in_=e100_ext[:])
        nc.sync.dma_start(out=e1[:], in_=e1_ext[:])

        io = ctx.enter_context(tc.tile_pool(name="io", bufs=IO_BUFS))
        wk = ctx.enter_context(tc.tile_pool(name="wk", bufs=WK_BUFS))
        ps1 = ctx.enter_context(tc.tile_pool(name="ps1", bufs=2,
                                             space="PSUM"))
        ps2 = ctx.enter_context(tc.tile_pool(name="ps2", bufs=2,
                                             space="PSUM"))
        psw = ctx.enter_context(tc.tile_pool(name="psw", bufs=1,
                                             space="PSUM"))

        # PE pstate warmup: ~3us of back-to-back matmuls on the weight
        # tiles; results never read
        warm = psw.tile([P, P], F32, name="warm")
        for _ in range(WARMUP_MM):
            nc.tensor.matmul(warm[:], e100[:], e1[:], start=True, stop=True)

        edr3 = edr[:].rearrange("p (t m) -> p t m", t=2)

        base = 0
        for t, G in enumerate(G_LIST):
            FD = G * SEQ
            F2 = G * LIMBS2
            F4 = G * LIMBS4
            rows = slice(base, base + P * G)
            base += P * G
            a_vt = a_ext[:][rows].rearrange("(p g) e -> p (g e)", p=P)
            b_vt = b_ext[:][rows].rearrange("(p g) e -> p (g e)", p=P)
            o_vt = o_ext[:][rows].rearrange("(p g) e -> p (g e)", p=P)

            a_t = io.tile([P, FD], FP8, tag="a", name=f"a_{t}",
                          padded_shape=[P, FDM])
            b_t = io.tile([P, FD], FP8, tag="b", name=f"b_{t}",
                          padded_shape=[P, FDM])
            nc.sync.dma_start(out=a_t[:], in_=a_vt)
            nc.sync.dma_start(out=b_t[:], in_=b_vt)

            # stage 1: base-100 limbs M = 10*(a+b)_hi + (a+b)_lo on PE.
            # rhs AP dims [p, t(k-tile), row, limb]: t picks the hi/lo
            # digit of each pair, limb stride -2 folds in the reversal.
            ps_t = ps1.tile([P, F2], F32, tag="ps1", name=f"ps1_{t}",
                            padded_shape=[P, F2M])
            A4 = a_t[:].rearrange("p (r m2 t) -> p t r m2",
                                  t=2, m2=LIMBS2)[:, :, :, ::-1]
            B4 = b_t[:].rearrange("p (r m2 t) -> p t r m2",
                                  t=2, m2=LIMBS2)[:, :, :, ::-1]
            W1 = min(MW, F2)
            RW1 = W1 // LIMBS2
            for h in range(F2 // W1):
                win = ps_t[:, h * W1:(h + 1) * W1]
                rs = slice(h * RW1, (h + 1) * RW1)
                nc.tensor.matmul(win, edr3, A4[:, :, rs], start=True,
                                 stop=False, perf_mode=DR)
                nc.tensor.matmul(win, edr3, B4[:, :, rs], start=False,
                                 stop=True, perf_mode=DR)

            # ACT drains M to SBUF bf16 (exact, M <= 198)
            m_t = wk.tile([P, F2], BF16, tag="m", name=f"m_{t}",
                          padded_shape=[P, F2M])
            nc.scalar.activation(m_t[:], ps_t[:],
                                 mybir.ActivationFunctionType.Copy)

            # stage 2: radix-10^4 limbs L = 100*M_odd + M_even on PE
            ps4_t = ps2.tile([P, F4], F32, tag="ps2", name=f"ps2_{t}",
                             padded_shape=[P, F4M])
            M3 = m_t[:].rearrange("p (r q t) -> p r q t", t=2, q=LIMBS4)
            W2 = min(MW, F4)
            RW2 = W2 // LIMBS4
            for h in range(F4 // W2):
                win = ps4_t[:, h * W2:(h + 1) * W2]
                rs = slice(h * RW2, (h + 1) * RW2)
                nc.tensor.matmul(win, e100[:], M3[:, rs, :, 1], start=True,
                                 stop=False)
                nc.tensor.matmul(win, e1[:], M3[:, rs, :, 0], start=False,
                                 stop=True)

            # whole carry chain: v_t = [10^4 <= v_{t-1}] + s_t, written
            # straight to the u16 output tile (v <= 19999, exact)
            d_t = wk.tile([P, F4], U16, tag="d", name=f"d_{t}",
                          padded_shape=[P, F4M])
            nc.vector.tensor_tensor_scan(
                out=d_t[:], data0=pat[:, 0:F4], data1=ps4_t[:],
                initial=0.0, op0=ALU.is_le, op1=ALU.add)

            nc.scalar.dma_start(out=o_vt, in_=d_t[:])

    nc.finalize()
    return nc


def _host_inputs(a, b):
    """Cast digit arrays to fp8 (exact for 0..9) and build per-core maps."""
    import ml_dtypes

    fp8 = ml_dtypes.float8_e4m3
    bf16 = ml_dtypes.bfloat16
    a8 = np.ascontiguousarray(np.asarray(a, dtype=np.float32)).astype(fp8)
    b8 = np.ascontiguousarray(np.asarray(b, dtype=np.float32)).astype(fp8)
    eye = np.eye(P, dtype=np.float32)
    eyedr = np.concatenate([10.0 * eye, eye], axis=1).astype(fp8)
    eye100 = (100.0 * eye).astype(bf16)
    eye1 = eye.astype(bf16)
    return [
        {"a": a8[i * B_LOC:(i + 1) * B_LOC],
         "b": b8[i * B_LOC:(i + 1) * B_LOC],
         "eyedr": eyedr, "eye100": eye100, "eye1": eye1}
        for i in range(N_CORES)
    ]


def _host_decode(results):
    """Concat per-core raw scan words (v = limb + 10^4*carry, LSB-first
    limb order) and decode into f32 digit columns."""
    raw = np.concatenate(
        [results[i]["out"] for i in range(N_CORES)], axis=0)  # (B, 16) u16
    v = (raw[:, ::-1] % 10000).astype(np.int32)
    out = np.empty((BATCH, SEQ), dtype=np.float32)
    q, out_3 = np.divmod(v, 10)
    q, out_2 = np.divmod(q, 10)
    out_0, out_1 = np.divmod(q, 10)
    out[:, 0::4] = out_0
    out[:, 1::4] = out_1
    out[:, 2::4] = out_2
    out[:, 3::4] = out_3
    return out


def kernel(a, b, weight_ih=None, weight_hh=None, bias_ih=None, bias_hh=None):
    """Full-batch digit adder. The RNN weights are the fixed carry-add
    weights baked into the module; the kernel implements that function
    directly, so they are accepted and unused."""
    from concourse.bass_utils import run_bass_kernel_spmd

    assert np.asarray(a).shape == (BATCH, SEQ)
    assert np.asarray(b).shape == (BATCH, SEQ)

    if "nc" not in _nc_cache:
        _nc_cache["nc"] = _build_adder()
    nc = _nc_cache["nc"]

    res = run_bass_kernel_spmd(nc, _host_inputs(a, b),
                               core_ids=list(range(N_CORES)))
    return _host_decode(res.results)


if __name__ == "__main__":
    rng = np.random.default_rng(0)
    a = rng.integers(0, 10, (BATCH, SEQ)).astype(np.float32)
    b = rng.integers(0, 10, (BATCH, SEQ)).astype(np.float32)
    out = kernel(a, b)
    # host reference
    c = np.zeros(BATCH, np.float32)
    exp = np.zeros_like(a)
    for e in range(SEQ - 1, -1, -1):
        s = a[:, e] + b[:, e] + c
        c = (s >= 10).astype(np.float32)
        exp[:, e] = s - 10 * c
    print("max abs err:", np.abs(out - exp).max())


# revision 11
# speedup vs baseline: 1.0686x; 1.0686x over previous
"""Trainium2 Bass kernel: batched 64-digit base-10 addition (nn_Adder).

The reference RNN scan is carry-propagating decimal addition. The DVE
scan instruction is the only engine that can walk the carry recurrence,
and it runs at a fixed ~2.15 ns/element regardless of dtype — so the
kernel works in radix-10^4: each group of FOUR digits is one limb,
quartering the scan length per row (16 limbs instead of 64 digits).

Per core (pure data parallel across 8 cores, batch 524288 -> 65536 rows):

  * Inputs are uploaded as fp8e4 (digits 0-9 exact in e4m3): 4x less
    input HBM traffic than f32. Output leaves as one uint16 per FOUR
    digits (the raw scan state v = limb + 10^4*carry <= 19999, exact in
    u16): 8x less output traffic. The host decodes v % 10^4 into digit
    columns with numpy divmods.
  * Stage 1 (PE, fp8 DoubleRow perf mode): one DoubleRow matmul per
    source computes 10*d_even + d_odd for every digit pair — the
    weights [10I | I] pair with strided views of the even/odd digit
    positions (k-tile dim), and the MSB->LSB reversal is folded into
    the same access pattern. PSUM gets base-100 limbs M <= 198,
    LSB-first.
  * ACT drains M to SBUF as bf16 (integers <= 256 exact in bf16).
  * Stage 2 (PE, bf16): two accumulating matmuls with weights 100I / I
    over the odd/even base-100 limbs -> PSUM radix-10^4 limbs <= 19998.
  * DVE runs the whole carry chain in ONE scan per tile, reading PSUM:
    v_t = [10^4 <= v_{t-1}] + s_t. Row boundaries are killed by the
    data0 pattern operand (3e9 at each row's first limb). The scan
    writes the u16 OUTPUT tile directly - zero post-processing.
  * A burst of dummy matmuls right after the weight DMAs ramps the PE
    pstate (full clock needs ~3us of continuous execution) before the
    first data tile lands.
  * Small first/last tiles (G_LIST) shorten pipeline fill and drain.
  * GpSimd untouched (it would steal the DVE SBUF ports).

All intermediate values are small integers, exact in fp8/bf16/f32/u16 ->
bit-exact output after the host-side decode.
"""

import sys

sys.path.insert(0, "/opt/trn_rl_repo")

import numpy as np

BATCH = 524288
SEQ = 64
N_CORES = 8
B_LOC = BATCH // N_CORES

P = 128
LIMBS2 = SEQ // 2   # 32 base-100 limbs per row (stage-1)
LIMBS4 = SEQ // 4   # 16 base-10^4 limbs per row (stage-2 / scan / output)
G_LIST = [16] + [32] * 15 + [16]
G_MAX = max(G_LIST)
T = len(G_LIST)
FDM = G_MAX * SEQ       # padded digit cols (a/b tiles)
F2M = G_MAX * LIMBS2    # padded base-100 limb cols
F4M = G_MAX * LIMBS4    # padded base-10^4 limb cols
MW = 512                # matmul window = one PSUM bank of f32

IO_BUFS = T             # all input DMAs queued up front
WK_BUFS = 4

_nc_cache = {}


def _build_adder():
    from contextlib import ExitStack

    import concourse.bacc as bacc
    import concourse.mybir as mybir
    import concourse.tile as tile

    F32 = mybir.dt.float32
    BF16 = mybir.dt.bfloat16
    FP8 = mybir.dt.float8e4
    U16 = mybir.dt.uint16
    ALU = mybir.AluOpType
    DR = mybir.MatmulPerfMode.DoubleRow

    assert P * sum(G_LIST) == B_LOC

    nc = bacc.Bacc("TRN2", target_bir_lowering=False, debug=False)
    a_ext = nc.declare_dram_parameter("a", [B_LOC, SEQ], FP8, isOutput=False)
    b_ext = nc.declare_dram_parameter("b", [B_LOC, SEQ], FP8, isOutput=False)
    # [10I | I] fp8 pair-weights for DoubleRow stage 1
    edr_ext = nc.declare_dram_parameter("eyedr", [P, 2 * P], FP8,
                                        isOutput=False)
    # 100I and I in bf16 for stage 2
    e100_ext = nc.declare_dram_parameter("eye100", [P, P], BF16,
                                         isOutput=False)
    e1_ext = nc.declare_dram_parameter("eye1", [P, P], BF16, isOutput=False)
    o_ext = nc.declare_dram_parameter("out", [B_LOC, LIMBS4], U16,
                                      isOutput=True)

    with tile.TileContext(nc) as tc, ExitStack() as ctx:
        cpool = ctx.enter_context(tc.tile_pool(name="const", bufs=1))
        io = ctx.enter_context(tc.tile_pool(name="io", bufs=IO_BUFS))
        wk = ctx.enter_context(tc.tile_pool(name="wk", bufs=WK_BUFS))
        ps1 = ctx.enter_context(tc.tile_pool(name="ps1", bufs=2,
                                             space="PSUM"))
        ps2 = ctx.enter_context(tc.tile_pool(name="ps2", bufs=2,
                                             space="PSUM"))

        # tile-0 input DMAs go FIRST: the body is DMA-ingest-paced, so
        # the first data transfer is the critical path, not the weights
        # (which are only needed once the data has landed).
        io_tiles = []
        base = 0
        for t, G in enumerate(G_LIST):
            rows = slice(base, base + P * G)
            base += P * G
            a_vt = a_ext[:][rows].rearrange("(p g) e -> p (g e)", p=P)
            b_vt = b_ext[:][rows].rearrange("(p g) e -> p (g e)", p=P)
            a_t = io.tile([P, G * SEQ], FP8, tag="a", name=f"a_{t}",
                          padded_shape=[P, FDM])
            b_t = io.tile([P, G * SEQ], FP8, tag="b", name=f"b_{t}",
                          padded_shape=[P, FDM])
            io_tiles.append((a_t, b_t))
            nc.sync.dma_start(out=a_t[:], in_=a_vt)
            nc.sync.dma_start(out=b_t[:], in_=b_vt)
            if t == 0:
                # weights + scan pattern, after the first data tile
                edr = cpool.tile([P, 2 * P], FP8)
                e100 = cpool.tile([P, P], BF16)
                e1 = cpool.tile([P, P], BF16)
                nc.sync.dma_start(out=edr[:], in_=edr_ext[:])
                nc.sync.dma_start(out=e100[:], in_=e100_ext[:])
                nc.sync.dma_start(out=e1[:], in_=e1_ext[:])
                pat = cpool.tile([P, F4M], F32)
                nc.vector.memset(pat[:], 10000.0)
                nc.vector.memset(pat[:, 0:F4M:LIMBS4], 3.0e9)

        edr3 = edr[:].rearrange("p (t m) -> p t m", t=2)

        base = 0
        for t, G in enumerate(G_LIST):
            FD = G * SEQ
            F2 = G * LIMBS2
            F4 = G * LIMBS4
            rows = slice(base, base + P * G)
            base += P * G
            o_vt = o_ext[:][rows].rearrange("(p g) e -> p (g e)", p=P)
            a_t, b_t = io_tiles[t]

            # stage 1: base-100 limbs M = 10*(a+b)_hi + (a+b)_lo on PE.
            # rhs AP dims [p, t(k-tile), row, limb]: t picks the hi/lo
            # digit of each pair, limb stride -2 folds in the reversal.
            ps_t = ps1.tile([P, F2], F32, tag="ps1", name=f"ps1_{t}",
                            padded_shape=[P, F2M])
            A4 = a_t[:].rearrange("p (r m2 t) -> p t r m2",
                                  t=2, m2=LIMBS2)[:, :, :, ::-1]
            B4 = b_t[:].rearrange("p (r m2 t) -> p t r m2",
                                  t=2, m2=LIMBS2)[:, :, :, ::-1]
            W1 = min(MW, F2)
            RW1 = W1 // LIMBS2
            for h in range(F2 // W1):
                win = ps_t[:, h * W1:(h + 1) * W1]
                rs = slice(h * RW1, (h + 1) * RW1)
                nc.tensor.matmul(win, edr3, A4[:, :, rs], start=True,
                                 stop=False, perf_mode=DR)
                nc.tensor.matmul(win, edr3, B4[:, :, rs], start=False,
                                 stop=True, perf_mode=DR)

            # ACT drains M to SBUF bf16 (exact, M <= 198)
            m_t = wk.tile([P, F2], BF16, tag="m", name=f"m_{t}",
                          padded_shape=[P, F2M])
            nc.scalar.activation(m_t[:], ps_t[:],
                                 mybir.ActivationFunctionType.Copy)

            # stage 2: radix-10^4 limbs L = 100*M_odd + M_even on PE
            ps4_t = ps2.tile([P, F4], F32, tag="ps2", name=f"ps2_{t}",
                             padded_shape=[P, F4M])
            M3 = m_t[:].rearrange("p (r q t) -> p r q t", t=2, q=LIMBS4)
            W2 = min(MW, F4)
            RW2 = W2 // LIMBS4
            for h in range(F4 // W2):
                win = ps4_t[:, h * W2:(h + 1) * W2]
                rs = slice(h * RW2, (h + 1) * RW2)
                nc.tensor.matmul(win, e100[:], M3[:, rs, :, 1], start=True,
                                 stop=False)
                nc.tensor.matmul(win, e1[:], M3[:, rs, :, 0], start=False,
                                 stop=True)

            # whole carry chain: v_t = [10^4 <= v_{t-1}] + s_t, written
            # straight to the u16 output tile (v <= 19999, exact)
            d_t = wk.tile([P, F4], U16, tag="d", name=f"d_{t}",
                          padded_shape=[P, F4M])
            nc.vector.tensor_tensor_scan(
                out=d_t[:], data0=pat[:, 0:F4], data1=ps4_t[:],
                initial=0.0, op0=ALU.is_le, op1=ALU.add)

            nc.scalar.dma_start(out=o_vt, in_=d_t[:])

    nc.finalize()
    return nc


def _host_inputs(a, b):
    """Cast digit arrays to fp8 (exact for 0..9) and build per-core maps."""
    import ml_dtypes

    fp8 = ml_dtypes.float8_e4m3
    bf16 = ml_dtypes.bfloat16
    a8 = np.ascontiguousarray(np.asarray(a, dtype=np.float32)).astype(fp8)
    b8 = np.ascontiguousarray(np.asarray(b, dtype=np.float32)).astype(fp8)
    eye = np.eye(P, dtype=np.float32)
    eyedr = np.concatenate([10.0 * eye, eye], axis=1).astype(fp8)
    eye100 = (100.0 * eye).astype(bf16)
    eye1 = eye.astype(bf16)
    return [
        {"a": a8[i * B_LOC:(i + 1) * B_LOC],
         "b": b8[i * B_LOC:(i + 1) * B_LOC],
         "eyedr": eyedr, "eye100": eye100, "eye1": eye1}
        for i in range(N_CORES)
    ]


def _host_decode(results):
    """Concat per-core raw scan words (v = limb + 10^4*carry, LSB-first
    limb order) and decode into f32 digit columns."""
    raw = np.concatenate(
        [results[i]["out"] for i in range(N_CORES)], axis=0)  # (B, 16) u16
    v = (raw[:, ::-1] % 10000).astype(np.int32)
    out = np.empty((BATCH, SEQ), dtype=np.float32)
    q, out_3 = np.divmod(v, 10)
    q, out_2 = np.divmod(q, 10)
    out_0, out_1 = np.divmod(q, 10)
    out[:, 0::4] = out_0
    out[:, 1::4] = out_1
    out[:, 2::4] = out_2
    out[:, 3::4] = out_3
    return out


def kernel(a, b, weight_ih=None, weight_hh=None, bias_ih=None, bias_hh=None):
    """Full-batch digit adder. The RNN weights are the fixed carry-add
    weights baked into the module; the kernel implements that function
    directly, so they are accepted and unused."""
    from concourse.bass_utils import run_bass_kernel_spmd

    assert np.asarray(a).shape == (BATCH, SEQ)
    assert np.asarray(b).shape == (BATCH, SEQ)

    if "nc" not in _nc_cache:
        _nc_cache["nc"] = _build_adder()
    nc = _nc_cache["nc"]

    res = run_bass_kernel_spmd(nc, _host_inputs(a, b),
                               core_ids=list(range(N_CORES)))
    return _host_decode(res.results)


if __name__ == "__main__":
    rng = np.random.default_rng(0)
    a = rng.integers(0, 10, (BATCH, SEQ)).astype(np.float32)
    b = rng.integers(0, 10, (BATCH, SEQ)).astype(np.float32)
    out = kernel(a, b)
    # host reference
    c = np.zeros(BATCH, np.float32)
    exp = np.zeros_like(a)
    for e in range(SEQ - 1, -1, -1):
        s = a[:, e] + b[:, e] + c
        c = (s >= 10).astype(np.float32)
        exp[:, e] = s - 10 * c
    print("max abs err:", np.abs(out - exp).max())
